# revision 26
# baseline (speedup 1.0000x reference)
"""Trainium2 Bass kernel for nn_Attention_Layer (dense transformer attention).

Computes, for X [N, D], Wq/Wk/Wv [D, D]:
    Q = X @ Wq.T ; K = X @ Wk.T ; V = X @ Wv.T
    O = softmax(Q @ K.T, axis=-1) @ V

Strategy (8 NeuronCores, SPMD single launch):
  - Shard rows of X across cores (N=8192 -> r=1024 rows/core).
  - Score reassociation: S = Q K^T = X (Wq^T Wk) X^T = A X^T with
    M = Wq^T Wk, A = X_b M.  This replaces the Q and K projections with a
    distributed M slice (each core computes a 128-col slice) + one A
    projection, and eliminates any K^T all-gather: the full X^T is an
    ExternalInput on every core, so score keys stream straight from DRAM.
  - Two small collectives remain: M slices (0.25 MB) and V_b (2 MB fp16),
    both all-gathered while unrelated PE work runs.
  - All stationary/moving matmul operands are fp16: 16-bit moving
    operands stream at ~2x the f32 rate on the TRN2 PE, weight loads are
    FWL-eligible (f32r is not), DRAM stream bytes halve vs f32, and the
    mantissa is ~8x finer than bf16 — total rel err ~2e-3 vs the fp64
    oracle.
  - PV accumulates each query subtile over SPAN=2 key blocks in a single
    PSUM group (one DVE drain per qs per span instead of per block), and
    row-sum groups open/close within one span — interleaving start=True
    into a PSUM bank that holds other OPEN accumulation groups corrupts
    them on hardware (learned the hard way).
  - Attention runs transposed: S^T[k, q] tiles from X^T chunks
    (stationary, streamed) x A^T (moving, resident); softmax uses a
    constant bias shift (exact after normalization); P~ = exp(S^T + bias)
    is stored bf16 (fp16 would over/underflow: logit rowmax spread ~49)
    and feeds P@V directly; row-sums come from tiny bf16 matmuls against
    a ones pair into a shared PSUM bank.  O accumulates in SBUF fp32; the
    final normalize + store is fused per query-subtile into the last key
    block so the tail overlaps PV compute.
  - Stage B is software-pipelined with an L-block lookahead: S^T/exp for
    block j+L is emitted alongside PV for block j, so the first PV (which
    needs all-gathered V rows) sits ~L*14us after stage-B start, hiding
    the V collective latency without any core-dependent addressing.
  - Per-block streams are single 3D DMAs ([128, DC, kb] for X^T keys,
    [128, KC, d] for V) instead of per-chunk transfers.

AllGather concatenates rank blocks on axis 0; key blocks are processed in
rank order on every core with the same (rank, local-row) indexing for X^T
keys and V rows, so the softmax/PV reduction is consistent.
"""

import numpy as np

import concourse.tile as tile
from concourse import bacc, mybir
from concourse.bass_utils import run_bass_kernel_spmd

N_CORES = 8
N_TOTAL = 8192
D_MODEL = 1024
R_PER_CORE = N_TOTAL // N_CORES  # 1024

F32 = mybir.dt.float32
F16 = mybir.dt.float16
BF16 = mybir.dt.bfloat16
EXP_BIAS = -45.0  # constant softmax shift; cancels exactly after normalization


def build_fused(
    n_cores=N_CORES,
    d=D_MODEL,
    r=R_PER_CORE,
    kb=512,
    exp_bias=EXP_BIAS,
    mock_ag=False,    # timing/sim builds: skip collectives, read own bounces
    repeat_attn=1,    # timing builds: run stage B this many times
    stream_bufs=3,    # buffering of streamed X^T/V tiles
    ps_a_bufs=4,      # stage-A psum pipelining depth (per tag)
    lookahead=4,      # S^T blocks emitted ahead of PV (V-gather slack)
    span=4,           # key blocks accumulated per PV psum group
    warmup_mms=10,    # PE p-state warmup matmuls
    split_dma=True,   # spread stage-A input loads across queues
    stage_b_mode="full",  # PROBE: "full" | "st" | "st_nocons" | "pv"
):
    """Build the fused M/A/V + AllGather + attention kernel (SPMD).

    Per-core I/O (all fp16):
      xt   [d, r]   ExternalInput — X^T columns for this core's rows
      xtf  [d, N]   ExternalInput — full X^T (replicated; score keys)
      wq   [d, d]   ExternalInput — Wq natural ([out, in]) (replicated)
      wko  [d, 128] ExternalInput — Wk natural cols for this core
      wvt  [d, d]   ExternalInput — Wv^T (replicated)
      o    [r, d]   ExternalOutput — this core's output rows
    """
    assert d % 128 == 0 and r % 128 == 0 and kb % 128 == 0
    DC = d // 128            # contraction chunks over d
    NQS = r // 128           # 128-query subtiles per core
    QG = min(512, r)         # query group (free dim) for S^T / A matmuls
    NQG = r // QG
    KC = kb // 128           # key chunks per key block
    BPR = r // kb            # key blocks per rank block
    DW = min(512, d)         # free-dim slice width over d
    ND = d // DW
    RW = min(512, r)
    NR = r // RW
    n_blocks = n_cores * BPR
    L = lookahead
    SPAN = span
    assert n_blocks % SPAN == 0

    nc = bacc.Bacc("TRN2", target_bir_lowering=False, debug=False, num_devices=n_cores)

    xt = nc.dram_tensor("xt", [d, r], F16, kind="ExternalInput").ap()
    xtf = nc.dram_tensor("xtf", [d, n_cores * r], F16, kind="ExternalInput").ap()
    wq = nc.dram_tensor("wq", [d, d], F16, kind="ExternalInput").ap()
    wko = nc.dram_tensor("wko", [d, 128], F16, kind="ExternalInput").ap()
    wvt = nc.dram_tensor("wvt", [d, d], F16, kind="ExternalInput").ap()
    o = nc.dram_tensor("o", [r, d], F16, kind="ExternalOutput").ap()

    # Internal DRAM bounces + gathers: M pair-slice and V_b (fp16).
    mb_ = nc.dram_tensor("mb", [d, 128], F16).ap()
    vb = nc.dram_tensor("vb", [r, d], F16).ap()
    mg = nc.dram_tensor("mg", [n_cores * d, 128], F16, addr_space="Shared").ap()
    vg = nc.dram_tensor("vg", [n_cores * r, d], F16, addr_space="Shared").ap()

    with tile.TileContext(nc) as tc:
        with tc.tile_pool(name="persist", bufs=1) as pp:
            # --- persistent tiles ---
            at_t = []
            for dc in range(DC):
                t = pp.tile([128, r], F16, name=f"at{dc}", tag=f"at{dc}")
                at_t.append(t)
            oacc = []
            for qs in range(NQS):
                t = pp.tile([128, d], F32, name=f"oacc{qs}", tag=f"oacc{qs}")
                oacc.append(t)
            oacc_rs = pp.tile([128, 2 * NQS], F32, name="oacc_rs", tag="oacc_rs")
            rsf_t = pp.tile([128, 2], F32, name="rsf_t", tag="rsf_t")
            ones_f32 = pp.tile([128, 2], F32, name="ones_f32", tag="ones_f32")
            nc.vector.memset(ones_f32, 1.0)
            ones_bf = pp.tile([128, 2], BF16, name="ones_bf", tag="ones_bf")
            nc.vector.tensor_copy(ones_bf, ones_f32)
            bias_t = pp.tile([128, 1], F32, name="bias_t", tag="bias_t")
            nc.vector.memset(bias_t, exp_bias)
            recip_t = pp.tile([128, NQS], F32, name="recip_t", tag="recip_t")

            # ---------------- Stage A: M, V, A ----------------
            # (the stage-B key stream pool wraps both stages so its buffers
            # don't alias stage-A tiles: the first xs blocks prefetch during
            # the A/V projections instead of waiting for the arena to free)
            ctx_xs = tc.tile_pool(name="xs_pool", bufs=stream_bufs)
            pxs = ctx_xs.__enter__()
            with (
                tc.tile_pool(name="stage_a", bufs=1) as pa,
                tc.tile_pool(name="ps_am", bufs=ps_a_bufs, space="PSUM") as ps_am,
                tc.tile_pool(name="ps_a", bufs=ps_a_bufs, space="PSUM") as ps_a,
                tc.tile_pool(name="outs_a", bufs=3) as pout_a,
                tc.tile_pool(name="mg_s", bufs=2) as pmg,
            ):
                eng = ([nc.sync, nc.gpsimd, nc.scalar]
                       if split_dma else [nc.sync, nc.sync])
                n_eng = len(eng)

                # PE p-state warmup: ~4us of throwaway matmuls on a zeroed
                # tile so the tensor clock is ramped when the first real
                # matmul's operands land.
                warm = pa.tile([128, 512], F16, name="warm", tag="warm")
                wz = pa.tile([128, 512], F32, name="wz", tag="wz")
                nc.vector.memset(wz, 0.0)
                nc.vector.tensor_copy(warm, wz)
                wps = ps_a.tile([128, 512], F32, name="wps", tag="ps")
                for wi in range(warmup_mms):
                    nc.tensor.matmul(wps, warm[:, 0:128], warm,
                                     start=(wi == 0), stop=(wi == warmup_mms - 1))

                # Loads: per-chunk DMAs (wq+wko first so the M-slice can
                # start, then xt column-halves, then wv column-halves).
                def ldc(e_idx, t, dram_rows, c0, c1):
                    eng[e_idx % n_eng].dma_start(
                        out=t[:, c0:c1], in_=dram_rows[:, c0:c1])

                wq_t, wko_t, xt_t, wv_t = [], [], [], []
                for oc in range(DC):
                    t = pa.tile([128, d], F16, name=f"wq{oc}", tag=f"wq{oc}")
                    wq_t.append(t)
                    t = pa.tile([128, 128], F16, name=f"wko{oc}", tag=f"wko{oc}")
                    ldc(oc, t, wko[oc * 128:(oc + 1) * 128, :], 0, 128)
                    wko_t.append(t)
                # wq in column-quarter waves: M-slice group i1c only needs
                # columns [i1c*128, i1c*128+128) of every row chunk, so the
                # first groups start as soon as the first waves land
                for q4 in range(4):
                    for oc in range(DC):
                        ldc(q4 * DC + oc, wq_t[oc],
                            wq[oc * 128:(oc + 1) * 128, :],
                            q4 * (d // 4), (q4 + 1) * (d // 4))
                for dc in range(DC):
                    t = pa.tile([128, r], F16, name=f"xt{dc}", tag=f"xt{dc}")
                    xt_t.append(t)
                    t = pa.tile([128, d], F16, name=f"wv{dc}", tag=f"wv{dc}")
                    wv_t.append(t)
                # first halves of xt and wv land before any second half: the
                # og=0 / rc<4 V-projection groups depend only on first halves
                for hh in range(2):
                    for dc in range(DC):
                        ldc(dc + hh, xt_t[dc], xt[dc * 128:(dc + 1) * 128, :],
                            hh * (r // 2), (hh + 1) * (r // 2))
                    for dc in range(DC):
                        ldc(dc + hh + 1, wv_t[dc],
                            wvt[dc * 128:(dc + 1) * 128, :],
                            hh * (d // 2), (hh + 1) * (d // 2))

                # M pair-slice: M[:, pair] = Wq^T @ Wk[:, pair]
                mo3 = pout_a.tile([128, DC, 128], F16, name="mo3",
                                  tag="mo3", bufs=1)
                for i1c in range(DC):
                    ps = ps_am.tile([128, 128], F32, name="psm", tag="psm")
                    for oc in range(DC):
                        nc.tensor.matmul(
                            ps,
                            wq_t[oc][:, i1c * 128:(i1c + 1) * 128],
                            wko_t[oc],
                            start=(oc == 0),
                            stop=(oc == DC - 1),
                        )
                    nc.vector.tensor_copy(mo3[:, i1c, :], ps)
                nc.sync.dma_start(
                    out=mb_.rearrange("(i p) j -> p i j", p=128), in_=mo3)
                if not mock_ag:
                    nc.gpsimd.collective_compute(
                        "AllGather",
                        mybir.AluOpType.bypass,
                        ins=[mb_],
                        outs=[mg],
                        replica_groups=[list(range(n_cores))],
                    )

                # V_b -> vb (fp16), then gather (hidden under A-proj + S^T)
                for og in range(ND):
                    for rc in range(r // 128):
                        ps = ps_a.tile([128, DW], F32, name="ps", tag="ps")
                        for dc in range(DC):
                            nc.tensor.matmul(
                                ps,
                                xt_t[dc][:, rc * 128:(rc + 1) * 128],
                                wv_t[dc][:, og * DW:(og + 1) * DW],
                                start=(dc == 0),
                                stop=(dc == DC - 1),
                            )
                        ot = pout_a.tile([128, DW], F16, name="vo", tag="vo")
                        nc.vector.tensor_copy(ot, ps)
                        nc.sync.dma_start(
                            out=vb[rc * 128:(rc + 1) * 128, og * DW:(og + 1) * DW],
                            in_=ot,
                        )
                if not mock_ag:
                    nc.gpsimd.collective_compute(
                        "AllGather",
                        mybir.AluOpType.bypass,
                        ins=[vb],
                        outs=[vg],
                        replica_groups=[list(range(n_cores))],
                    )

                # A^T = M^T X^T kept in SBUF (its PE work hides both gathers)
                # whole gathered M in one DMA; rg-outer loop order so the
                # first query half of every at_t chunk (what stage B's first
                # S^T groups read) is complete before the second half starts
                mgall = pmg.tile([128, DC * DC, 128], F16, name="mgall",
                                 tag="mgall", bufs=1)
                if mock_ag:
                    for oc in range(DC):
                        nc.sync.dma_start(
                            out=mgall[:, oc * DC:(oc + 1) * DC, :],
                            in_=mb_.rearrange("(dc p) j -> p dc j", p=128))
                else:
                    nc.sync.dma_start(
                        out=mgall,
                        in_=mg.rearrange("(c p) j -> p c j", p=128))
                for rg in range(NR):
                    for oc in range(DC):
                        ps = ps_a.tile([128, RW], F32, name="ps", tag="ps")
                        for dc in range(DC):
                            nc.tensor.matmul(
                                ps,
                                mgall[:, oc * DC + dc, :],
                                xt_t[dc][:, rg * RW:(rg + 1) * RW],
                                start=(dc == 0),
                                stop=(dc == DC - 1),
                            )
                        nc.vector.tensor_copy(
                            at_t[oc][:, rg * RW:(rg + 1) * RW], ps)

            # ---------------- Stage B: attention (L-lookahead pipeline) ----
            with (
                tc.tile_pool(name="v_pool", bufs=span + 1) as pv8,
                tc.tile_pool(name="pt_pool", bufs=L + SPAN) as ppt,
                tc.tile_pool(name="ps_st", bufs=3, space="PSUM") as ps_st,
                tc.tile_pool(name="ps_pv", bufs=2, space="PSUM") as ps_pv,
                tc.tile_pool(name="ps_rs", bufs=1, space="PSUM") as ps_rs,
                tc.tile_pool(name="outp", bufs=3) as pout,
            ):
                xtf3 = xtf.rearrange("(c p) n -> p c n", p=128)
                vg3 = (vb if mock_ag else vg).rearrange("(c p) j -> p c j", p=128)
                n_iters = repeat_attn * n_blocks

                def do_st(st_i):
                    """S^T + exp for key block st_i%n_blocks -> pt tile dict."""
                    blk = st_i % n_blocks
                    k0 = blk * kb
                    xs = pxs.tile([128, DC, kb], F16, name="xs", tag="xs")
                    nc.sync.dma_start(out=xs, in_=xtf3[:, :, k0:k0 + kb])
                    pt_t = {}
                    for kc in range(KC):
                        for qg in range(NQG):
                            ps = ps_st.tile([128, QG], F32, name="st_ps",
                                            tag="st_ps")
                            for dc in range(DC):
                                nc.tensor.matmul(
                                    ps,
                                    xs[:, dc, kc * 128:(kc + 1) * 128],
                                    at_t[dc][:, qg * QG:(qg + 1) * QG],
                                    start=(dc == 0),
                                    stop=(dc == DC - 1),
                                )
                            if stage_b_mode == "st_nocons":
                                continue
                            pt = ppt.tile([128, QG], BF16, name="pt",
                                          tag=f"pt{kc}_{qg}")
                            nc.scalar.activation(
                                pt, ps, mybir.ActivationFunctionType.Exp,
                                bias=bias_t, scale=1.0,
                            )
                            pt_t[(kc, qg)] = pt
                    return pt_t

                def do_pv(spn, pt_list):
                    """P@V + row-sums for SPAN key blocks starting at block
                    (spn*SPAN)%n_blocks. O accumulates over the span in a
                    single psum group per (query subtile, d-half) — one DVE
                    drain per qs per span instead of per block; row-sums
                    accumulate per span and fold into oacc_rs on DVE."""
                    spn0 = spn % (n_blocks // SPAN)
                    first = spn0 == 0
                    last = spn0 == n_blocks // SPAN - 1
                    vts = []
                    for s in range(SPAN):
                        blk = (spn0 * SPAN + s) % n_blocks
                        if mock_ag:
                            half = blk % BPR
                            vsl = vg3[:, half * KC:(half + 1) * KC, :]
                        else:
                            vsl = vg3[:, blk * KC:(blk + 1) * KC, :]
                        vt = pv8.tile([128, KC, d], F16, name="vt", tag="vt")
                        nc.gpsimd.dma_start(out=vt, in_=vsl)
                        vts.append(vt)

                    rs = ps_rs.tile([128, 2 * NQS], F32, name="rs_ps",
                                    tag="rs_ps")
                    for qs in range(NQS):
                        qg, off = divmod(qs * 128, QG)
                        pv = [
                            ps_pv.tile([128, DW], F32, name="pv_ps",
                                       tag=f"pv{nd}")
                            for nd in range(ND)
                        ]
                        for s in range(SPAN):
                            for kc in range(KC):
                                lhsT = pt_list[s][(kc, qg)][:, off:off + 128]
                                for nd in range(ND):
                                    nc.tensor.matmul(
                                        pv[nd],
                                        lhsT,
                                        vts[s][:, kc, nd * DW:(nd + 1) * DW],
                                        start=(s == 0 and kc == 0),
                                        stop=(s == SPAN - 1 and kc == KC - 1),
                                        skip_group_check=True,
                                    )
                                nc.tensor.matmul(
                                    rs[:, 2 * qs:2 * qs + 2],
                                    lhsT,
                                    ones_bf,
                                    start=(s == 0 and kc == 0),
                                    stop=(s == SPAN - 1 and kc == KC - 1),
                                    skip_group_check=True,
                                )
                        if first:
                            for nd in range(ND):
                                nc.vector.tensor_copy(
                                    oacc[qs][:, nd * DW:(nd + 1) * DW], pv[nd])
                        elif not last:
                            for nd in range(ND):
                                nc.vector.tensor_add(
                                    oacc[qs][:, nd * DW:(nd + 1) * DW],
                                    oacc[qs][:, nd * DW:(nd + 1) * DW], pv[nd])
                        else:
                            # fused tail: finalize this query subtile now so
                            # normalize/store overlap the remaining PV work
                            nc.vector.tensor_add(
                                rsf_t, oacc_rs[:, 2 * qs:2 * qs + 2],
                                rs[:, 2 * qs:2 * qs + 2])
                            nc.vector.reciprocal(
                                recip_t[:, qs:qs + 1], rsf_t[:, 0:1])
                            ot = pout.tile([128, d], F32, name="ot", tag="ot")
                            ob = pout.tile([128, d], F16, name="ob", tag="ob")
                            for nd in range(ND):
                                sl = slice(nd * DW, (nd + 1) * DW)
                                nc.vector.tensor_add(
                                    ot[:, sl], oacc[qs][:, sl], pv[nd])
                                nc.vector.tensor_scalar_mul(
                                    ob[:, sl], ot[:, sl], recip_t[:, qs:qs + 1])
                            nc.sync.dma_start(
                                out=o[qs * 128:(qs + 1) * 128, :], in_=ob)
                    if first:
                        nc.vector.tensor_copy(oacc_rs, rs)
                    elif not last:
                        nc.vector.tensor_add(oacc_rs, oacc_rs, rs)

                if stage_b_mode in ("st", "st_nocons"):
                    for i in range(n_iters):
                        do_st(i)
                elif stage_b_mode == "pv":
                    pt0 = do_st(0)
                    for spn in range(n_iters // SPAN):
                        do_pv(spn, [pt0] * SPAN)
                else:
                    # S^T runs L blocks ahead; PV follows in SPAN-block
                    # groups once its span of pt tiles is complete
                    pt_ring = {}
                    n_spans = n_iters // SPAN
                    for i in range(n_iters + L + SPAN - 1):
                        if i < n_iters:
                            pt_ring[i] = do_st(i)
                        j = i - L - SPAN + 1
                        if j >= 0 and j % SPAN == 0:
                            do_pv(j // SPAN,
                                  [pt_ring.pop(j + s) for s in range(SPAN)])
            ctx_xs.__exit__(None, None, None)

    nc.compile()
    return nc


_NC_CACHE = {}


def _get_nc():
    if "fused" not in _NC_CACHE:
        _NC_CACHE["fused"] = build_fused()
    return _NC_CACHE["fused"]


def make_in_maps(X, Wq, Wk, Wv, n_cores=N_CORES, r=R_PER_CORE):
    X = np.ascontiguousarray(np.asarray(X, dtype=np.float32))
    XTh = np.ascontiguousarray(X.T).astype(np.float16)
    Wqh = np.ascontiguousarray(np.asarray(Wq, dtype=np.float32)).astype(np.float16)
    Wkh = np.ascontiguousarray(np.asarray(Wk, dtype=np.float32)).astype(np.float16)
    WvTh = np.ascontiguousarray(
        np.asarray(Wv, dtype=np.float32).T).astype(np.float16)
    maps = []
    for c in range(n_cores):
        maps.append({
            "xt": np.ascontiguousarray(XTh[:, c * r:(c + 1) * r]),
            "xtf": XTh,
            "wq": Wqh,
            "wko": np.ascontiguousarray(Wkh[:, c * 128:(c + 1) * 128]),
            "wvt": WvTh,
        })
    return maps


def kernel(inputs, Wq, Wk, Wv):
    nc = _get_nc()
    in_maps = make_in_maps(inputs, Wq, Wk, Wv)
    res = run_bass_kernel_spmd(nc, in_maps, core_ids=list(range(N_CORES)))
    out = np.concatenate(
        [np.asarray(res.results[c]["o"]) for c in range(N_CORES)], axis=0)
    return out.astype(np.float32)


# revision 27
# speedup vs baseline: 1.3892x; 1.3892x over previous
"""Trainium2 Bass kernel for nn_Attention_Layer (dense transformer attention).

Computes, for X [N, D], Wq/Wk/Wv [D, D]:
    Q = X @ Wq.T ; K = X @ Wk.T ; V = X @ Wv.T
    O = softmax(Q @ K.T, axis=-1) @ V

Strategy (8 NeuronCores, SPMD single launch):
  - Shard rows of X across cores (N=8192 -> r=1024 rows/core).
  - Score reassociation: S = Q K^T = X (Wq^T Wk) X^T = A X^T with
    M = Wq^T Wk, A = X_b M.  This replaces the Q and K projections with a
    distributed M slice (each core computes a 128-col slice) + one A
    projection, and eliminates any K^T all-gather: the full X^T is an
    ExternalInput on every core, so score keys stream straight from DRAM.
  - Two small collectives remain: M slices (0.25 MB) and V_b (2 MB fp16),
    both all-gathered while unrelated PE work runs.
  - All stationary/moving matmul operands are fp16: 16-bit moving
    operands stream at ~2x the f32 rate on the TRN2 PE, weight loads are
    FWL-eligible (f32r is not), DRAM stream bytes halve vs f32, and the
    mantissa is ~8x finer than bf16 — total rel err ~2e-3 vs the fp64
    oracle.
  - PV accumulates each query subtile over SPAN=2 key blocks in a single
    PSUM group (one DVE drain per qs per span instead of per block), and
    row-sum groups open/close within one span — interleaving start=True
    into a PSUM bank that holds other OPEN accumulation groups corrupts
    them on hardware (learned the hard way).
  - Attention runs transposed: S^T[k, q] tiles from X^T chunks
    (stationary, streamed) x A^T (moving, resident); softmax uses a
    constant bias shift (exact after normalization); P~ = exp(S^T + bias)
    is stored bf16 (fp16 would over/underflow: logit rowmax spread ~49)
    and feeds P@V directly; row-sums come from tiny bf16 matmuls against
    a ones pair into a shared PSUM bank.  O accumulates in SBUF fp32; the
    final normalize + store is fused per query-subtile into the last key
    block so the tail overlaps PV compute.
  - Stage B is software-pipelined with an L-block lookahead: S^T/exp for
    block j+L is emitted alongside PV for block j, so the first PV (which
    needs all-gathered V rows) sits ~L*14us after stage-B start, hiding
    the V collective latency without any core-dependent addressing.
  - Per-block streams are single 3D DMAs ([128, DC, kb] for X^T keys,
    [128, KC, d] for V) instead of per-chunk transfers.

AllGather concatenates rank blocks on axis 0; key blocks are processed in
rank order on every core with the same (rank, local-row) indexing for X^T
keys and V rows, so the softmax/PV reduction is consistent.
"""

import numpy as np

import concourse.tile as tile
from concourse import bacc, mybir
from concourse.bass_utils import run_bass_kernel_spmd

N_CORES = 8
N_TOTAL = 8192
D_MODEL = 1024
R_PER_CORE = N_TOTAL // N_CORES  # 1024

F32 = mybir.dt.float32
F16 = mybir.dt.float16
BF16 = mybir.dt.bfloat16
EXP_BIAS = -45.0  # constant softmax shift; cancels exactly after normalization


def build_fused(
    n_cores=N_CORES,
    d=D_MODEL,
    r=R_PER_CORE,
    kb=512,
    exp_bias=EXP_BIAS,
    mock_ag=False,    # timing/sim builds: skip collectives, read own bounces
    repeat_attn=1,    # timing builds: run stage B this many times
    stream_bufs=3,    # buffering of streamed X^T/V tiles
    ps_a_bufs=4,      # stage-A psum pipelining depth (per tag)
    lookahead=6,      # S^T blocks emitted ahead of PV (V-gather slack)
    span=2,           # key blocks accumulated per PV psum group
    warmup_mms=10,    # PE p-state warmup matmuls
    split_dma=True,   # spread stage-A input loads across queues
    stage_b_mode="full",  # PROBE: "full" | "st" | "st_nocons" | "pv"
):
    """Build the fused M/A/V + AllGather + attention kernel (SPMD).

    Per-core I/O (all fp16):
      xt   [d, r]   ExternalInput — X^T columns for this core's rows
      xtf  [d, N]   ExternalInput — full X^T (replicated; score keys)
      wq   [d, d]   ExternalInput — Wq natural ([out, in]) (replicated)
      wko  [d, 128] ExternalInput — Wk natural cols for this core
      wvt  [d, d]   ExternalInput — Wv^T (replicated)
      o    [r, d]   ExternalOutput — this core's output rows
    """
    assert d % 128 == 0 and r % 128 == 0 and kb % 128 == 0
    DC = d // 128            # contraction chunks over d
    NQS = r // 128           # 128-query subtiles per core
    QG = min(512, r)         # query group (free dim) for S^T / A matmuls
    NQG = r // QG
    KC = kb // 128           # key chunks per key block
    BPR = r // kb            # key blocks per rank block
    DW = min(512, d)         # free-dim slice width over d
    ND = d // DW
    RW = min(512, r)
    NR = r // RW
    n_blocks = n_cores * BPR
    L = lookahead
    SPAN = span
    assert n_blocks % SPAN == 0

    nc = bacc.Bacc("TRN2", target_bir_lowering=False, debug=False, num_devices=n_cores)

    xt = nc.dram_tensor("xt", [d, r], F16, kind="ExternalInput").ap()
    xtf = nc.dram_tensor("xtf", [d, n_cores * r], F16, kind="ExternalInput").ap()
    wq = nc.dram_tensor("wq", [d, d], F16, kind="ExternalInput").ap()
    wko = nc.dram_tensor("wko", [d, 128], F16, kind="ExternalInput").ap()
    wvt = nc.dram_tensor("wvt", [d, d], F16, kind="ExternalInput").ap()
    o = nc.dram_tensor("o", [r, d], F16, kind="ExternalOutput").ap()

    # Internal DRAM bounces + gathers: M pair-slice and V_b (fp16).
    mb_ = nc.dram_tensor("mb", [d, 128], F16).ap()
    vb = nc.dram_tensor("vb", [r, d], F16).ap()
    mg = nc.dram_tensor("mg", [n_cores * d, 128], F16, addr_space="Shared").ap()
    vg = nc.dram_tensor("vg", [n_cores * r, d], F16, addr_space="Shared").ap()

    with tile.TileContext(nc) as tc:
        with tc.tile_pool(name="persist", bufs=1) as pp:
            # --- persistent tiles ---
            at_t = []
            for dc in range(DC):
                t = pp.tile([128, r], F16, name=f"at{dc}", tag=f"at{dc}")
                at_t.append(t)
            oacc = []
            for qs in range(NQS):
                t = pp.tile([128, d], F32, name=f"oacc{qs}", tag=f"oacc{qs}")
                oacc.append(t)
            oacc_rs = pp.tile([128, 2 * NQS], F32, name="oacc_rs", tag="oacc_rs")
            rsf_t = pp.tile([128, 2], F32, name="rsf_t", tag="rsf_t")
            ones_f32 = pp.tile([128, 2], F32, name="ones_f32", tag="ones_f32")
            nc.vector.memset(ones_f32, 1.0)
            ones_bf = pp.tile([128, 2], BF16, name="ones_bf", tag="ones_bf")
            nc.vector.tensor_copy(ones_bf, ones_f32)
            bias_t = pp.tile([128, 1], F32, name="bias_t", tag="bias_t")
            nc.vector.memset(bias_t, exp_bias)
            recip_t = pp.tile([128, NQS], F32, name="recip_t", tag="recip_t")

            # ---------------- Stage A: M, V, A ----------------
            # (the stage-B key stream pool wraps both stages so its buffers
            # don't alias stage-A tiles: the first xs blocks prefetch during
            # the A/V projections instead of waiting for the arena to free)
            ctx_xs = tc.tile_pool(name="xs_pool", bufs=stream_bufs)
            pxs = ctx_xs.__enter__()
            with (
                tc.tile_pool(name="stage_a", bufs=1) as pa,
                tc.tile_pool(name="ps_am", bufs=ps_a_bufs, space="PSUM") as ps_am,
                tc.tile_pool(name="ps_a", bufs=ps_a_bufs, space="PSUM") as ps_a,
                tc.tile_pool(name="outs_a", bufs=3) as pout_a,
                tc.tile_pool(name="mg_s", bufs=2) as pmg,
            ):
                eng = ([nc.sync, nc.gpsimd, nc.scalar]
                       if split_dma else [nc.sync, nc.sync])
                n_eng = len(eng)

                # PE p-state warmup: ~4us of throwaway matmuls on a zeroed
                # tile so the tensor clock is ramped when the first real
                # matmul's operands land.
                warm = pa.tile([128, 512], F16, name="warm", tag="warm")
                wz = pa.tile([128, 512], F32, name="wz", tag="wz")
                nc.vector.memset(wz, 0.0)
                nc.vector.tensor_copy(warm, wz)
                wps = ps_a.tile([128, 512], F32, name="wps", tag="ps")
                for wi in range(warmup_mms):
                    nc.tensor.matmul(wps, warm[:, 0:128], warm,
                                     start=(wi == 0), stop=(wi == warmup_mms - 1))

                # Loads: per-chunk DMAs (wq+wko first so the M-slice can
                # start, then xt column-halves, then wv column-halves).
                def ldc(e_idx, t, dram_rows, c0, c1):
                    eng[e_idx % n_eng].dma_start(
                        out=t[:, c0:c1], in_=dram_rows[:, c0:c1])

                wq_t, wko_t, xt_t, wv_t = [], [], [], []
                for oc in range(DC):
                    t = pa.tile([128, d], F16, name=f"wq{oc}", tag=f"wq{oc}")
                    wq_t.append(t)
                    t = pa.tile([128, 128], F16, name=f"wko{oc}", tag=f"wko{oc}")
                    ldc(oc, t, wko[oc * 128:(oc + 1) * 128, :], 0, 128)
                    wko_t.append(t)
                # wq in column-quarter waves: M-slice group i1c only needs
                # columns [i1c*128, i1c*128+128) of every row chunk, so the
                # first groups start as soon as the first waves land
                for q4 in range(4):
                    for oc in range(DC):
                        ldc(q4 * DC + oc, wq_t[oc],
                            wq[oc * 128:(oc + 1) * 128, :],
                            q4 * (d // 4), (q4 + 1) * (d // 4))
                for dc in range(DC):
                    t = pa.tile([128, r], F16, name=f"xt{dc}", tag=f"xt{dc}")
                    xt_t.append(t)
                    t = pa.tile([128, d], F16, name=f"wv{dc}", tag=f"wv{dc}")
                    wv_t.append(t)
                # first halves of xt and wv land before any second half: the
                # og=0 / rc<4 V-projection groups depend only on first halves
                for hh in range(2):
                    for dc in range(DC):
                        ldc(dc + hh, xt_t[dc], xt[dc * 128:(dc + 1) * 128, :],
                            hh * (r // 2), (hh + 1) * (r // 2))
                    for dc in range(DC):
                        ldc(dc + hh + 1, wv_t[dc],
                            wvt[dc * 128:(dc + 1) * 128, :],
                            hh * (d // 2), (hh + 1) * (d // 2))

                # M pair-slice: M[:, pair] = Wq^T @ Wk[:, pair]
                mo3 = pout_a.tile([128, DC, 128], F16, name="mo3",
                                  tag="mo3", bufs=1)
                for i1c in range(DC):
                    ps = ps_am.tile([128, 128], F32, name="psm", tag="psm")
                    for oc in range(DC):
                        nc.tensor.matmul(
                            ps,
                            wq_t[oc][:, i1c * 128:(i1c + 1) * 128],
                            wko_t[oc],
                            start=(oc == 0),
                            stop=(oc == DC - 1),
                        )
                    nc.vector.tensor_copy(mo3[:, i1c, :], ps)
                nc.sync.dma_start(
                    out=mb_.rearrange("(i p) j -> p i j", p=128), in_=mo3)
                if not mock_ag:
                    nc.gpsimd.collective_compute(
                        "AllGather",
                        mybir.AluOpType.bypass,
                        ins=[mb_],
                        outs=[mg],
                        replica_groups=[list(range(n_cores))],
                    )

                # V_b -> vb (fp16), then gather (hidden under A-proj + S^T)
                for og in range(ND):
                    for rc in range(r // 128):
                        ps = ps_a.tile([128, DW], F32, name="ps", tag="ps")
                        for dc in range(DC):
                            nc.tensor.matmul(
                                ps,
                                xt_t[dc][:, rc * 128:(rc + 1) * 128],
                                wv_t[dc][:, og * DW:(og + 1) * DW],
                                start=(dc == 0),
                                stop=(dc == DC - 1),
                            )
                        ot = pout_a.tile([128, DW], F16, name="vo", tag="vo")
                        nc.vector.tensor_copy(ot, ps)
                        nc.sync.dma_start(
                            out=vb[rc * 128:(rc + 1) * 128, og * DW:(og + 1) * DW],
                            in_=ot,
                        )
                if not mock_ag:
                    nc.gpsimd.collective_compute(
                        "AllGather",
                        mybir.AluOpType.bypass,
                        ins=[vb],
                        outs=[vg],
                        replica_groups=[list(range(n_cores))],
                    )

                # A^T = M^T X^T kept in SBUF (its PE work hides both gathers)
                # whole gathered M in one DMA; rg-outer loop order so the
                # first query half of every at_t chunk (what stage B's first
                # S^T groups read) is complete before the second half starts
                mgall = pmg.tile([128, DC * DC, 128], F16, name="mgall",
                                 tag="mgall", bufs=1)
                if mock_ag:
                    for oc in range(DC):
                        nc.sync.dma_start(
                            out=mgall[:, oc * DC:(oc + 1) * DC, :],
                            in_=mb_.rearrange("(dc p) j -> p dc j", p=128))
                else:
                    nc.sync.dma_start(
                        out=mgall,
                        in_=mg.rearrange("(c p) j -> p c j", p=128))
                for rg in range(NR):
                    for oc in range(DC):
                        ps = ps_a.tile([128, RW], F32, name="ps", tag="ps")
                        for dc in range(DC):
                            nc.tensor.matmul(
                                ps,
                                mgall[:, oc * DC + dc, :],
                                xt_t[dc][:, rg * RW:(rg + 1) * RW],
                                start=(dc == 0),
                                stop=(dc == DC - 1),
                            )
                        nc.vector.tensor_copy(
                            at_t[oc][:, rg * RW:(rg + 1) * RW], ps)

            # ---------------- Stage B: attention (L-lookahead pipeline) ----
            with (
                tc.tile_pool(name="v_pool", bufs=span + 1) as pv8,
                tc.tile_pool(name="pt_pool", bufs=L + SPAN) as ppt,
                tc.tile_pool(name="ps_st", bufs=3, space="PSUM") as ps_st,
                tc.tile_pool(name="ps_pv", bufs=2, space="PSUM") as ps_pv,
                tc.tile_pool(name="ps_rs", bufs=1, space="PSUM") as ps_rs,
                tc.tile_pool(name="outp", bufs=3) as pout,
            ):
                xtf3 = xtf.rearrange("(c p) n -> p c n", p=128)
                vg3 = (vb if mock_ag else vg).rearrange("(c p) j -> p c j", p=128)
                n_iters = repeat_attn * n_blocks

                def do_st(st_i):
                    """S^T + exp for key block st_i%n_blocks -> pt tile dict."""
                    blk = st_i % n_blocks
                    k0 = blk * kb
                    xs = pxs.tile([128, DC, kb], F16, name="xs", tag="xs")
                    nc.sync.dma_start(out=xs, in_=xtf3[:, :, k0:k0 + kb])
                    pt_t = {}
                    for kc in range(KC):
                        for qg in range(NQG):
                            ps = ps_st.tile([128, QG], F32, name="st_ps",
                                            tag="st_ps")
                            for dc in range(DC):
                                nc.tensor.matmul(
                                    ps,
                                    xs[:, dc, kc * 128:(kc + 1) * 128],
                                    at_t[dc][:, qg * QG:(qg + 1) * QG],
                                    start=(dc == 0),
                                    stop=(dc == DC - 1),
                                )
                            if stage_b_mode == "st_nocons":
                                continue
                            pt = ppt.tile([128, QG], BF16, name="pt",
                                          tag=f"pt{kc}_{qg}")
                            nc.scalar.activation(
                                pt, ps, mybir.ActivationFunctionType.Exp,
                                bias=bias_t, scale=1.0,
                            )
                            pt_t[(kc, qg)] = pt
                    return pt_t

                def do_pv(spn, pt_list):
                    """P@V + row-sums for SPAN key blocks starting at block
                    (spn*SPAN)%n_blocks. O accumulates over the span in a
                    single psum group per (query subtile, d-half) — one DVE
                    drain per qs per span instead of per block; row-sums
                    accumulate per span and fold into oacc_rs on DVE."""
                    spn0 = spn % (n_blocks // SPAN)
                    first = spn0 == 0
                    last = spn0 == n_blocks // SPAN - 1
                    vts = []
                    for s in range(SPAN):
                        blk = (spn0 * SPAN + s) % n_blocks
                        if mock_ag:
                            half = blk % BPR
                            vsl = vg3[:, half * KC:(half + 1) * KC, :]
                        else:
                            vsl = vg3[:, blk * KC:(blk + 1) * KC, :]
                        vt = pv8.tile([128, KC, d], F16, name="vt", tag="vt")
                        nc.gpsimd.dma_start(out=vt, in_=vsl)
                        vts.append(vt)

                    rs = ps_rs.tile([128, 2 * NQS], F32, name="rs_ps",
                                    tag="rs_ps")
                    for qs in range(NQS):
                        qg, off = divmod(qs * 128, QG)
                        pv = [
                            ps_pv.tile([128, DW], F32, name="pv_ps",
                                       tag=f"pv{nd}")
                            for nd in range(ND)
                        ]
                        for s in range(SPAN):
                            for kc in range(KC):
                                lhsT = pt_list[s][(kc, qg)][:, off:off + 128]
                                for nd in range(ND):
                                    nc.tensor.matmul(
                                        pv[nd],
                                        lhsT,
                                        vts[s][:, kc, nd * DW:(nd + 1) * DW],
                                        start=(s == 0 and kc == 0),
                                        stop=(s == SPAN - 1 and kc == KC - 1),
                                        skip_group_check=True,
                                    )
                                nc.tensor.matmul(
                                    rs[:, 2 * qs:2 * qs + 2],
                                    lhsT,
                                    ones_bf,
                                    start=(s == 0 and kc == 0),
                                    stop=(s == SPAN - 1 and kc == KC - 1),
                                    skip_group_check=True,
                                )
                        if first:
                            for nd in range(ND):
                                nc.vector.tensor_copy(
                                    oacc[qs][:, nd * DW:(nd + 1) * DW], pv[nd])
                        elif not last:
                            for nd in range(ND):
                                nc.vector.tensor_add(
                                    oacc[qs][:, nd * DW:(nd + 1) * DW],
                                    oacc[qs][:, nd * DW:(nd + 1) * DW], pv[nd])
                        else:
                            # fused tail: finalize this query subtile now so
                            # normalize/store overlap the remaining PV work
                            nc.vector.tensor_add(
                                rsf_t, oacc_rs[:, 2 * qs:2 * qs + 2],
                                rs[:, 2 * qs:2 * qs + 2])
                            nc.vector.reciprocal(
                                recip_t[:, qs:qs + 1], rsf_t[:, 0:1])
                            ot = pout.tile([128, d], F32, name="ot", tag="ot")
                            ob = pout.tile([128, d], F16, name="ob", tag="ob")
                            for nd in range(ND):
                                sl = slice(nd * DW, (nd + 1) * DW)
                                nc.vector.tensor_add(
                                    ot[:, sl], oacc[qs][:, sl], pv[nd])
                                nc.vector.tensor_scalar_mul(
                                    ob[:, sl], ot[:, sl], recip_t[:, qs:qs + 1])
                            nc.sync.dma_start(
                                out=o[qs * 128:(qs + 1) * 128, :], in_=ob)
                    if first:
                        nc.vector.tensor_copy(oacc_rs, rs)
                    elif not last:
                        nc.vector.tensor_add(oacc_rs, oacc_rs, rs)

                if stage_b_mode in ("st", "st_nocons"):
                    for i in range(n_iters):
                        do_st(i)
                elif stage_b_mode == "pv":
                    pt0 = do_st(0)
                    for spn in range(n_iters // SPAN):
                        do_pv(spn, [pt0] * SPAN)
                else:
                    # S^T runs L blocks ahead; PV follows in SPAN-block
                    # groups once its span of pt tiles is complete
                    pt_ring = {}
                    n_spans = n_iters // SPAN
                    for i in range(n_iters + L + SPAN - 1):
                        if i < n_iters:
                            pt_ring[i] = do_st(i)
                        j = i - L - SPAN + 1
                        if j >= 0 and j % SPAN == 0:
                            do_pv(j // SPAN,
                                  [pt_ring.pop(j + s) for s in range(SPAN)])
            ctx_xs.__exit__(None, None, None)

    nc.compile()
    return nc


_NC_CACHE = {}


def _get_nc():
    if "fused" not in _NC_CACHE:
        _NC_CACHE["fused"] = build_fused()
    return _NC_CACHE["fused"]


def make_in_maps(X, Wq, Wk, Wv, n_cores=N_CORES, r=R_PER_CORE):
    X = np.ascontiguousarray(np.asarray(X, dtype=np.float32))
    XTh = np.ascontiguousarray(X.T).astype(np.float16)
    Wqh = np.ascontiguousarray(np.asarray(Wq, dtype=np.float32)).astype(np.float16)
    Wkh = np.ascontiguousarray(np.asarray(Wk, dtype=np.float32)).astype(np.float16)
    WvTh = np.ascontiguousarray(
        np.asarray(Wv, dtype=np.float32).T).astype(np.float16)
    maps = []
    for c in range(n_cores):
        maps.append({
            "xt": np.ascontiguousarray(XTh[:, c * r:(c + 1) * r]),
            "xtf": XTh,
            "wq": Wqh,
            "wko": np.ascontiguousarray(Wkh[:, c * 128:(c + 1) * 128]),
            "wvt": WvTh,
        })
    return maps


def kernel(inputs, Wq, Wk, Wv):
    nc = _get_nc()
    in_maps = make_in_maps(inputs, Wq, Wk, Wv)
    res = run_bass_kernel_spmd(nc, in_maps, core_ids=list(range(N_CORES)))
    out = np.concatenate(
        [np.asarray(res.results[c]["o"]) for c in range(N_CORES)], axis=0)
    return out.astype(np.float32)
